# revision 13
# baseline (speedup 1.0000x reference)
"""MoE layer (8 experts, top-2) Trainium2 kernel.

Strategy (expert-parallel, per sharding hint): the host computes the tiny
router (logits -> top-2 -> softmax gates), then dispatches tokens to the 8
NeuronCores by selected expert -- core e holds expert e's weights resident in
SBUF and runs the dense FFN over the tokens routed to it.  Each token-expert
contribution is gate-scaled on device; the host scatter-adds the two
contributions per token back into the full output.

Device kernel per core (SPMD, same NEFF, different data):
    hT = gelu(w1.T @ xT + b1)        (mm1: lhsT=w1 chunk, rhs=xT chunk)
    y  = hT.T @ w2 + 0               (mm2: lhsT=hT chunk, rhs=w2 chunk)
    out = y * gate                   (per-token gate, DVE tensor_scalar)

All matmuls are bf16 inputs with fp32 PSUM accumulation.
"""

import numpy as np
import ml_dtypes

P = 128
D_MODEL = 1024
D_FF = 4096
NUM_EXPERTS = 8
KT1 = D_MODEL // P   # 8  k-tiles for mm1
KT2 = D_FF // P      # 32 k-tiles for mm2
FFC = D_FF // P      # 32 ff chunks (mm1 output partition tiles)
N_OUT_HALF = D_MODEL // 2  # 512, free dim of mm2 matmuls

BF16 = ml_dtypes.bfloat16

_NC_CACHE: dict = {}
LAST_RESULTS = None  # BassKernelResults of the most recent device run


def _build(chunks, act="Gelu"):
    """Build + compile the per-core FFN kernel for the given token chunking."""
    import concourse.bacc as bacc
    import concourse.tile as tile
    import concourse.mybir as mybir

    n_pad = sum(chunks)
    bf16 = mybir.dt.bfloat16
    f32 = mybir.dt.float32

    nc = bacc.Bacc("TRN2", target_bir_lowering=False, debug=False)
    xT_d = nc.dram_tensor("xT", [D_MODEL, n_pad], bf16, kind="ExternalInput").ap()
    w1_d = nc.dram_tensor("w1", [D_MODEL, D_FF], bf16, kind="ExternalInput").ap()
    w2_d = nc.dram_tensor("w2", [D_FF, D_MODEL], bf16, kind="ExternalInput").ap()
    b1_d = nc.dram_tensor("b1", [P, FFC], f32, kind="ExternalInput").ap()
    g_d = nc.dram_tensor("g", [P, n_pad // P], f32, kind="ExternalInput").ap()
    out_d = nc.dram_tensor("out", [n_pad, D_MODEL], f32, kind="ExternalOutput").ap()

    xT_r = xT_d.rearrange("(ko p) t -> p ko t", p=P)   # [128, 8, n_pad]
    w1_r = w1_d.rearrange("(ko p) f -> p ko f", p=P)   # [128, 8, 4096]
    w2_r = w2_d.rearrange("(ko p) d -> p ko d", p=P)   # [128, 32, 1024]

    GELU = getattr(mybir.ActivationFunctionType, act)

    with tile.TileContext(nc) as tc:
        with (
            tc.tile_pool(name="wpool", bufs=1) as wpool,
            tc.tile_pool(name="xpool", bufs=2) as xpool,
            tc.tile_pool(name="hpool", bufs=1) as hpool,
            tc.tile_pool(name="opool", bufs=4) as opool,
            tc.tile_pool(name="php", bufs=4, space="PSUM") as php,
            tc.tile_pool(name="pyp", bufs=4, space="PSUM") as pyp,
        ):
            # First supertile's tokens and w1 land chunk-by-chunk, interleaved,
            # in mm1's consumption order, so the PE starts after ~2 MB instead
            # of waiting for the full 9.5 MB weight+token load.  w1 is split
            # into per-(k-tile, ff-slice) tiles; mm1 only waits on the slice
            # it reads.
            W1SL = 512  # ff columns per w1 slice (4 ffc chunks)
            n_sl = D_FF // W1SL
            xt0 = xpool.tile([P, KT1, 512], bf16, tag="xt")
            w1_sb = [[None] * n_sl for _ in range(KT1)]
            for kt in range(KT1):
                nc.sync.dma_start(
                    xt0[:, kt, : chunks[0]], xT_r[:, kt, : chunks[0]]
                )
                t = wpool.tile([P, W1SL], bf16, tag=f"w1_{kt}_0")
                nc.sync.dma_start(t[:], w1_r[:, kt, 0:W1SL])
                w1_sb[kt][0] = t
            for sl in range(1, n_sl):
                for kt in range(KT1):
                    t = wpool.tile([P, W1SL], bf16, tag=f"w1_{kt}_{sl}")
                    nc.sync.dma_start(
                        t[:], w1_r[:, kt, sl * W1SL : (sl + 1) * W1SL]
                    )
                    w1_sb[kt][sl] = t
            # b1/g are tiny and first consumed ~55 us in (after mm1 of the
            # first supertile); keep them off the critical DMA path.
            b1_sb = wpool.tile([P, FFC], f32, tag="b1")
            nc.sync.dma_start(b1_sb[:], b1_d[:])
            g_sb = wpool.tile([P, n_pad // P], f32, tag="g")
            nc.sync.dma_start(g_sb[:], g_d[:])
            w2_sb = []
            for kt in range(KT2):
                t = wpool.tile([P, D_MODEL], bf16, tag=f"w2_{kt}")
                nc.sync.dma_start(t[:], w2_r[:, kt, :])
                w2_sb.append(t)

            tok0 = 0
            for st, tok_len in enumerate(chunks):
                if st == 0:
                    xt = xt0
                else:
                    xt = xpool.tile([P, KT1, 512], bf16, tag="xt")
                    nc.sync.dma_start(
                        xt[:, :, :tok_len], xT_r[:, :, tok0 : tok0 + tok_len]
                    )
                ht = hpool.tile([P, KT2, 512], bf16, tag="ht")
                for ffc in range(FFC):
                    ph = php.tile([P, 512], f32, tag="ph")
                    sl, col = divmod(ffc * P, W1SL)
                    for kt in range(KT1):
                        nc.tensor.matmul(
                            ph[:, :tok_len],
                            w1_sb[kt][sl][:, col : col + P],
                            xt[:, kt, :tok_len],
                            start=(kt == 0),
                            stop=(kt == KT1 - 1),
                        )
                    nc.scalar.activation(
                        ht[:, ffc, :tok_len],
                        ph[:, :tok_len],
                        GELU,
                        bias=b1_sb[:, ffc : ffc + 1],
                        scale=1.0,
                    )
                for mt in range(tok_len // P):
                    gcol = tok0 // P + mt
                    py0 = pyp.tile([P, N_OUT_HALF], f32, tag="py")
                    py1 = pyp.tile([P, N_OUT_HALF], f32, tag="py")
                    for kt in range(KT2):
                        lhsT = ht[:, kt, mt * P : (mt + 1) * P]
                        nc.tensor.matmul(
                            py0, lhsT, w2_sb[kt][:, 0:N_OUT_HALF],
                            start=(kt == 0), stop=(kt == KT2 - 1),
                        )
                        nc.tensor.matmul(
                            py1, lhsT, w2_sb[kt][:, N_OUT_HALF:D_MODEL],
                            start=(kt == 0), stop=(kt == KT2 - 1),
                        )
                    for nb, py in ((0, py0), (1, py1)):
                        ot = opool.tile([P, N_OUT_HALF], f32, tag="ot")
                        nc.vector.tensor_scalar_mul(
                            ot[:], py[:], g_sb[:, gcol : gcol + 1]
                        )
                        nc.sync.dma_start(
                            out_d[
                                tok0 + mt * P : tok0 + (mt + 1) * P,
                                nb * N_OUT_HALF : (nb + 1) * N_OUT_HALF,
                            ],
                            ot[:],
                        )
                tok0 += tok_len
    nc.compile()
    return nc


def _get_nc(chunks):
    key = tuple(chunks)
    if key not in _NC_CACHE:
        _NC_CACHE[key] = _build(chunks)
    return _NC_CACHE[key]


def kernel(x, router_w, router_b, w1, b1, w2, b2):
    from concourse.bass_utils import run_bass_kernel_spmd

    x = np.asarray(x, dtype=np.float32)
    router_w = np.asarray(router_w, dtype=np.float32)
    router_b = np.asarray(router_b, dtype=np.float32)
    w1 = np.asarray(w1, dtype=np.float32)
    b1 = np.asarray(b1, dtype=np.float32)
    w2 = np.asarray(w2, dtype=np.float32)
    b2 = np.asarray(b2, dtype=np.float32)

    B, S, D = x.shape
    T = B * S
    xf = x.reshape(T, D)

    # --- host router: top-2 + softmax gates (tiny: T x D x 8) ---
    logits = xf @ router_w + router_b                      # [T, 8] fp32
    sel0 = np.argmax(logits, axis=1)
    l0 = logits[np.arange(T), sel0]
    masked = logits.copy()
    masked[np.arange(T), sel0] = -np.inf
    sel1 = np.argmax(masked, axis=1)
    l1 = masked[np.arange(T), sel1]
    # softmax over the two selected logits (l0 >= l1)
    e1 = np.exp(l1 - l0)
    g0 = 1.0 / (1.0 + e1)
    g1 = e1 / (1.0 + e1)

    # --- dispatch: gather tokens per expert ---
    token_ids = []
    gate_per = []
    for e in range(NUM_EXPERTS):
        m0 = sel0 == e
        m1 = sel1 == e
        ids = np.nonzero(m0 | m1)[0]
        g = np.where(m0, g0, 0.0) + np.where(m1, g1, 0.0)
        token_ids.append(ids)
        gate_per.append(g[ids].astype(np.float32))

    max_cnt = max(len(ids) for ids in token_ids)
    n_pad = max(P, ((max_cnt + P - 1) // P) * P)
    chunks = []
    rem = n_pad
    while rem > 0:
        c = min(512, rem)
        chunks.append(c)
        rem -= c

    nc = _get_nc(chunks)

    w1_bf = w1.astype(BF16)   # [8, 1024, 4096]
    w2_bf = w2.astype(BF16)   # [8, 4096, 1024]

    in_maps = []
    for e in range(NUM_EXPERTS):
        ids = token_ids[e]
        cnt = len(ids)
        xT = np.zeros((D_MODEL, n_pad), dtype=BF16)
        xT[:, :cnt] = np.ascontiguousarray(xf[ids].T).astype(BF16)
        gpad = np.zeros((n_pad,), dtype=np.float32)
        gpad[:cnt] = gate_per[e]
        in_maps.append(
            {
                "xT": xT,
                "w1": np.ascontiguousarray(w1_bf[e]),
                "w2": np.ascontiguousarray(w2_bf[e]),
                "b1": np.ascontiguousarray(b1[e].reshape(FFC, P).T),
                "g": np.ascontiguousarray(gpad.reshape(n_pad // P, P).T),
            }
        )

    try:
        res = run_bass_kernel_spmd(nc, in_maps, core_ids=list(range(NUM_EXPERTS)))
    except Exception:
        # Transient device errors (e.g. NRT_EXEC_UNIT_UNRECOVERABLE from a
        # wedged core) usually clear on a fresh attempt.
        res = run_bass_kernel_spmd(nc, in_maps, core_ids=list(range(NUM_EXPERTS)))
    global LAST_RESULTS
    LAST_RESULTS = res

    out = np.zeros((T, D), dtype=np.float32)
    for e in range(NUM_EXPERTS):
        ids = token_ids[e]
        out[ids] += res.results[e]["out"][: len(ids)]
    if b2.any():
        out += g0[:, None] * b2[sel0] + g1[:, None] * b2[sel1]
    return out.reshape(B, S, D)


# revision 15
# speedup vs baseline: 1.0358x; 1.0358x over previous
"""MoE layer (8 experts, top-2) Trainium2 kernel.

Strategy (expert-parallel, per sharding hint): the host computes the tiny
router (logits -> top-2 -> softmax gates), then dispatches tokens to the 8
NeuronCores by selected expert -- core e holds expert e's weights resident in
SBUF and runs the dense FFN over the tokens routed to it.  Each token-expert
contribution is gate-scaled on device; the host scatter-adds the two
contributions per token back into the full output.

Device kernel per core (SPMD, same NEFF, different data):
    hT = gelu(w1.T @ xT + b1)        (mm1: lhsT=w1 chunk, rhs=xT chunk)
    y  = hT.T @ w2 + 0               (mm2: lhsT=hT chunk, rhs=w2 chunk)
    out = y * gate                   (per-token gate, DVE tensor_scalar)

All matmuls are bf16 inputs with fp32 PSUM accumulation.
"""

import numpy as np
import ml_dtypes

P = 128
D_MODEL = 1024
D_FF = 4096
NUM_EXPERTS = 8
KT1 = D_MODEL // P   # 8  k-tiles for mm1
KT2 = D_FF // P      # 32 k-tiles for mm2
FFC = D_FF // P      # 32 ff chunks (mm1 output partition tiles)
N_OUT_HALF = D_MODEL // 2  # 512, free dim of mm2 matmuls

BF16 = ml_dtypes.bfloat16

_NC_CACHE: dict = {}
LAST_RESULTS = None  # BassKernelResults of the most recent device run


def _build(chunks, act="Gelu"):
    """Build + compile the per-core FFN kernel for the given token chunking."""
    import concourse.bacc as bacc
    import concourse.tile as tile
    import concourse.mybir as mybir

    n_pad = sum(chunks)
    bf16 = mybir.dt.bfloat16
    f32 = mybir.dt.float32

    nc = bacc.Bacc("TRN2", target_bir_lowering=False, debug=False)
    xT_d = nc.dram_tensor("xT", [D_MODEL, n_pad], bf16, kind="ExternalInput").ap()
    w1_d = nc.dram_tensor("w1", [D_MODEL, D_FF], bf16, kind="ExternalInput").ap()
    w2_d = nc.dram_tensor("w2", [D_FF, D_MODEL], bf16, kind="ExternalInput").ap()
    b1_d = nc.dram_tensor("b1", [P, FFC], f32, kind="ExternalInput").ap()
    g_d = nc.dram_tensor("g", [P, n_pad // P], f32, kind="ExternalInput").ap()
    out_d = nc.dram_tensor("out", [n_pad, D_MODEL], f32, kind="ExternalOutput").ap()

    xT_r = xT_d.rearrange("(ko p) t -> p ko t", p=P)   # [128, 8, n_pad]
    w1_r = w1_d.rearrange("(ko p) f -> p ko f", p=P)   # [128, 8, 4096]
    w2_r = w2_d.rearrange("(ko p) d -> p ko d", p=P)   # [128, 32, 1024]

    GELU = getattr(mybir.ActivationFunctionType, act)

    with tile.TileContext(nc) as tc:
        with (
            tc.tile_pool(name="wpool", bufs=1) as wpool,
            tc.tile_pool(name="xpool", bufs=2) as xpool,
            tc.tile_pool(name="hpool", bufs=1) as hpool,
            tc.tile_pool(name="opool", bufs=4) as opool,
            tc.tile_pool(name="php", bufs=4, space="PSUM") as php,
            tc.tile_pool(name="pyp", bufs=4, space="PSUM") as pyp,
        ):
            # b1/g are tiny but b1 gates the FIRST gelu (which drains the mm1
            # PSUM pool) -- they must land before the weight stream or the PE
            # stalls on PSUM slots once 4 groups complete.
            b1_sb = wpool.tile([P, FFC], f32, tag="b1")
            nc.sync.dma_start(b1_sb[:], b1_d[:])
            g_sb = wpool.tile([P, n_pad // P], f32, tag="g")
            nc.sync.dma_start(g_sb[:], g_d[:])

            # First supertile's tokens and w1 land chunk-by-chunk, interleaved,
            # in mm1's consumption order, so the PE starts after ~2 MB instead
            # of waiting for the full 9.5 MB weight+token load.  w1 is split
            # into per-(k-tile, ff-slice) tiles; mm1 only waits on the slice
            # it reads.
            W1SL = 512  # ff columns per w1 slice (4 ffc chunks)
            n_sl = D_FF // W1SL
            xt0 = xpool.tile([P, KT1, 512], bf16, tag="xt")
            w1_sb = [[None] * n_sl for _ in range(KT1)]
            for kt in range(KT1):
                nc.sync.dma_start(
                    xt0[:, kt, : chunks[0]], xT_r[:, kt, : chunks[0]]
                )
                t = wpool.tile([P, W1SL], bf16, tag=f"w1_{kt}_0")
                nc.sync.dma_start(t[:], w1_r[:, kt, 0:W1SL])
                w1_sb[kt][0] = t
            for sl in range(1, n_sl):
                for kt in range(KT1):
                    t = wpool.tile([P, W1SL], bf16, tag=f"w1_{kt}_{sl}")
                    nc.sync.dma_start(
                        t[:], w1_r[:, kt, sl * W1SL : (sl + 1) * W1SL]
                    )
                    w1_sb[kt][sl] = t
            w2_sb = []
            for kt in range(KT2):
                t = wpool.tile([P, D_MODEL], bf16, tag=f"w2_{kt}")
                nc.sync.dma_start(t[:], w2_r[:, kt, :])
                w2_sb.append(t)

            tok0 = 0
            for st, tok_len in enumerate(chunks):
                if st == 0:
                    xt = xt0
                else:
                    xt = xpool.tile([P, KT1, 512], bf16, tag="xt")
                    nc.sync.dma_start(
                        xt[:, :, :tok_len], xT_r[:, :, tok0 : tok0 + tok_len]
                    )
                ht = hpool.tile([P, KT2, 512], bf16, tag="ht")
                for ffc in range(FFC):
                    ph = php.tile([P, 512], f32, tag="ph")
                    sl, col = divmod(ffc * P, W1SL)
                    for kt in range(KT1):
                        nc.tensor.matmul(
                            ph[:, :tok_len],
                            w1_sb[kt][sl][:, col : col + P],
                            xt[:, kt, :tok_len],
                            start=(kt == 0),
                            stop=(kt == KT1 - 1),
                        )
                    nc.scalar.activation(
                        ht[:, ffc, :tok_len],
                        ph[:, :tok_len],
                        GELU,
                        bias=b1_sb[:, ffc : ffc + 1],
                        scale=1.0,
                    )
                for mt in range(tok_len // P):
                    gcol = tok0 // P + mt
                    py0 = pyp.tile([P, N_OUT_HALF], f32, tag="py")
                    py1 = pyp.tile([P, N_OUT_HALF], f32, tag="py")
                    for kt in range(KT2):
                        lhsT = ht[:, kt, mt * P : (mt + 1) * P]
                        nc.tensor.matmul(
                            py0, lhsT, w2_sb[kt][:, 0:N_OUT_HALF],
                            start=(kt == 0), stop=(kt == KT2 - 1),
                        )
                        nc.tensor.matmul(
                            py1, lhsT, w2_sb[kt][:, N_OUT_HALF:D_MODEL],
                            start=(kt == 0), stop=(kt == KT2 - 1),
                        )
                    for nb, py in ((0, py0), (1, py1)):
                        ot = opool.tile([P, N_OUT_HALF], f32, tag="ot")
                        nc.vector.tensor_scalar_mul(
                            ot[:], py[:], g_sb[:, gcol : gcol + 1]
                        )
                        nc.sync.dma_start(
                            out_d[
                                tok0 + mt * P : tok0 + (mt + 1) * P,
                                nb * N_OUT_HALF : (nb + 1) * N_OUT_HALF,
                            ],
                            ot[:],
                        )
                tok0 += tok_len
    nc.compile()
    return nc


def _get_nc(chunks):
    key = tuple(chunks)
    if key not in _NC_CACHE:
        _NC_CACHE[key] = _build(chunks)
    return _NC_CACHE[key]


def kernel(x, router_w, router_b, w1, b1, w2, b2):
    from concourse.bass_utils import run_bass_kernel_spmd

    x = np.asarray(x, dtype=np.float32)
    router_w = np.asarray(router_w, dtype=np.float32)
    router_b = np.asarray(router_b, dtype=np.float32)
    w1 = np.asarray(w1, dtype=np.float32)
    b1 = np.asarray(b1, dtype=np.float32)
    w2 = np.asarray(w2, dtype=np.float32)
    b2 = np.asarray(b2, dtype=np.float32)

    B, S, D = x.shape
    T = B * S
    xf = x.reshape(T, D)

    # --- host router: top-2 + softmax gates (tiny: T x D x 8) ---
    logits = xf @ router_w + router_b                      # [T, 8] fp32
    sel0 = np.argmax(logits, axis=1)
    l0 = logits[np.arange(T), sel0]
    masked = logits.copy()
    masked[np.arange(T), sel0] = -np.inf
    sel1 = np.argmax(masked, axis=1)
    l1 = masked[np.arange(T), sel1]
    # softmax over the two selected logits (l0 >= l1)
    e1 = np.exp(l1 - l0)
    g0 = 1.0 / (1.0 + e1)
    g1 = e1 / (1.0 + e1)

    # --- dispatch: gather tokens per expert ---
    token_ids = []
    gate_per = []
    for e in range(NUM_EXPERTS):
        m0 = sel0 == e
        m1 = sel1 == e
        ids = np.nonzero(m0 | m1)[0]
        g = np.where(m0, g0, 0.0) + np.where(m1, g1, 0.0)
        token_ids.append(ids)
        gate_per.append(g[ids].astype(np.float32))

    max_cnt = max(len(ids) for ids in token_ids)
    n_pad = max(P, ((max_cnt + P - 1) // P) * P)
    chunks = []
    rem = n_pad
    while rem > 0:
        c = min(512, rem)
        chunks.append(c)
        rem -= c

    nc = _get_nc(chunks)

    w1_bf = w1.astype(BF16)   # [8, 1024, 4096]
    w2_bf = w2.astype(BF16)   # [8, 4096, 1024]

    in_maps = []
    for e in range(NUM_EXPERTS):
        ids = token_ids[e]
        cnt = len(ids)
        xT = np.zeros((D_MODEL, n_pad), dtype=BF16)
        xT[:, :cnt] = np.ascontiguousarray(xf[ids].T).astype(BF16)
        gpad = np.zeros((n_pad,), dtype=np.float32)
        gpad[:cnt] = gate_per[e]
        in_maps.append(
            {
                "xT": xT,
                "w1": np.ascontiguousarray(w1_bf[e]),
                "w2": np.ascontiguousarray(w2_bf[e]),
                "b1": np.ascontiguousarray(b1[e].reshape(FFC, P).T),
                "g": np.ascontiguousarray(gpad.reshape(n_pad // P, P).T),
            }
        )

    try:
        res = run_bass_kernel_spmd(nc, in_maps, core_ids=list(range(NUM_EXPERTS)))
    except Exception:
        # Transient device errors (e.g. NRT_EXEC_UNIT_UNRECOVERABLE from a
        # wedged core) usually clear on a fresh attempt.
        res = run_bass_kernel_spmd(nc, in_maps, core_ids=list(range(NUM_EXPERTS)))
    global LAST_RESULTS
    LAST_RESULTS = res

    out = np.zeros((T, D), dtype=np.float32)
    for e in range(NUM_EXPERTS):
        ids = token_ids[e]
        out[ids] += res.results[e]["out"][: len(ids)]
    if b2.any():
        out += g0[:, None] * b2[sel0] + g1[:, None] * b2[sel1]
    return out.reshape(B, S, D)
